# revision 15
# baseline (speedup 1.0000x reference)
"""Trainium2 Bass kernel for fused cross-adjacency:
    w = einsum('m,mtd->td', head_w, mats); z = w @ x.T + head_b
    out = where(sigmoid(z) < 0.1, 0, sigmoid(z))           # [T=64, N=100000]

Sharding: node dim N split across 8 cores (12500 nodes each); tiny params
replicated. Host feeds x pre-transposed ([D=128, N/8] per core) so the
contraction dim D lands on SBUF partitions with no on-chip transpose.

Per chunk pair (2 x s columns): one input DMA brings [128, 2s] of xT; two
col-tiled matmuls (out partitions 0:64 / 64:128 of one PSUM bank) compute z
for both chunks; ScalarE applies sigmoid(z + b) with the bias folded into
the activation; VectorE applies the prune (sig >= 0.1) * sig in one
scalar_tensor_tensor; one output DMA stores the packed [128, s] tile.
Output DRAM is a packed [128, 6250] layout (two T=64 row-halves per column
block), unpacked on host. Raw Bass with a 4-slot ring pipeline: input DMA
on the SP queue, output DMA on the Activation HWDGE queue, so input/output
transfers ride different queues.
"""

import contextlib
import numpy as np

import concourse.bass as bass
import concourse.mybir as mybir
from concourse.bass_utils import run_bass_kernel_spmd

N, T, D, M = 100000, 64, 128, 8
N_CORES = 8
NSH = N // N_CORES  # 12500
CROSS_PRUNE = 0.1

# pair p processes two consecutive chunks of s columns each; chunk A goes to
# packed rows 0:64, chunk B to rows 64:128, at packed columns [poff, poff+s).
PAIR_SIZES = [500] * 12 + [250]
PACKED_W = sum(PAIR_SIZES)  # 6250
assert 2 * PACKED_W == NSH

SLOTS = 4  # ring depth for xt / z / sig / adj
SLOT_W = max(PAIR_SIZES)

F32 = mybir.dt.float32
NPAIR = len(PAIR_SIZES)


def build_nc():
    nc = bass.Bass()
    xT = nc.declare_dram_parameter("xT", [D, NSH], F32, isOutput=False)
    matsT = nc.declare_dram_parameter("matsT", [M, D, T], F32, isOutput=False)
    # [head_w(8), head_b(1), ones(128)] in one row
    headwb = nc.declare_dram_parameter("headwb", [1, M + 1 + D], F32, isOutput=False)
    out = nc.declare_dram_parameter("out", [D, PACKED_W], F32, isOutput=True)

    ctx = contextlib.ExitStack()
    with ctx:
        hwb = ctx.enter_context(nc.sbuf_tensor("hwb", [1, M + 1 + D], F32))
        bc = ctx.enter_context(nc.sbuf_tensor("bc", [D, M + 1], F32))
        mats_sb = ctx.enter_context(nc.sbuf_tensor("mats_sb", [D, M * T], F32))
        w0 = ctx.enter_context(nc.sbuf_tensor("w0", [D, T], F32))
        w1 = ctx.enter_context(nc.sbuf_tensor("w1", [D, T], F32))
        xt = [
            ctx.enter_context(nc.sbuf_tensor(f"xt{i}", [D, 2 * SLOT_W], F32))
            for i in range(SLOTS)
        ]
        sig = [
            ctx.enter_context(nc.sbuf_tensor(f"sig{i}", [D, SLOT_W], F32))
            for i in range(SLOTS)
        ]
        adj = [
            ctx.enter_context(nc.sbuf_tensor(f"adj{i}", [D, SLOT_W], F32))
            for i in range(SLOTS)
        ]
        bc_ps = ctx.enter_context(nc.psum_tensor("bc_ps", [D, M + 1], F32))
        z = [
            ctx.enter_context(nc.psum_tensor(f"z{i}", [D, SLOT_W], F32))
            for i in range(SLOTS)
        ]

        s_hwb = ctx.enter_context(nc.semaphore("s_hwb"))
        s_mats = ctx.enter_context(nc.semaphore("s_mats"))
        s_pe_pre = ctx.enter_context(nc.semaphore("s_pe_pre"))
        s_bc = ctx.enter_context(nc.semaphore("s_bc"))
        s_w = ctx.enter_context(nc.semaphore("s_w"))
        s_x = ctx.enter_context(nc.semaphore("s_x"))
        s_mm = ctx.enter_context(nc.semaphore("s_mm"))
        s_sig = ctx.enter_context(nc.semaphore("s_sig"))
        s_adjv = ctx.enter_context(nc.semaphore("s_adjv"))
        s_out = ctx.enter_context(nc.semaphore("s_out"))

        wacc = [w0, w1]
        wT = wacc[(M - 1) % 2]

        xoffs = []
        poffs = []
        xo = po = 0
        for s in PAIR_SIZES:
            xoffs.append(xo)
            poffs.append(po)
            xo += 2 * s
            po += s

        block = ctx.enter_context(nc.Block())

        @block.sync
        def _(sync):
            sync.dma_start(out=hwb[:, :], in_=headwb[:, :]).then_inc(s_hwb, 16)
            for m in range(M):
                sync.dma_start(
                    out=mats_sb[:, m * T : (m + 1) * T], in_=matsT[m, :, :]
                ).then_inc(s_mats, 16)
            for p, s in enumerate(PAIR_SIZES):
                if p >= SLOTS:
                    # PE must be done reading xt slot (mm2 of pair p-SLOTS)
                    sync.wait_ge(s_mm, 2 * (p - SLOTS) + 2)
                sync.dma_start(
                    out=xt[p % SLOTS][:, 0 : 2 * s],
                    in_=xT[:, xoffs[p] : xoffs[p] + 2 * s],
                ).then_inc(s_x, 16)

        @block.tensor
        def _(pe):
            pe.wait_ge(s_hwb, 16)
            # broadcast head_w/head_b to all 128 partitions: ones^T @ [hw|hb]
            pe.matmul(
                bc_ps[:, :], hwb[:, M + 1 :], hwb[:, 0 : M + 1],
                start=True, stop=True,
            ).then_inc(s_pe_pre, 1)
            pe.wait_ge(s_w, 1)
            for p, s in enumerate(PAIR_SIZES):
                pe.wait_ge(s_x, 16 * (p + 1))
                if p >= SLOTS:
                    # ACT must be done reading z slot (sigmoid of pair p-SLOTS)
                    pe.wait_ge(s_sig, p - SLOTS + 1)
                zz = z[p % SLOTS]
                xx = xt[p % SLOTS]
                pe.matmul(
                    zz[0:T, 0:s], wT[:, :], xx[:, 0:s], start=True, stop=True
                ).then_inc(s_mm, 1)
                pe.matmul(
                    zz[T:D, 0:s], wT[:, :], xx[:, s : 2 * s],
                    start=True, stop=True,
                ).then_inc(s_mm, 1)

        @block.vector
        def _(dve):
            dve.wait_ge(s_pe_pre, 1)
            dve.tensor_copy(bc[:, :], bc_ps[:, :]).then_inc(s_bc, 1)
            dve.wait_ge(s_mats, 16 * M)
            # wT[d, t] = sum_m head_w[m] * matsT[m, d, t]
            dve.tensor_scalar(
                wacc[0][:, :], mats_sb[:, 0:T], bc[:, 0:1], None,
                mybir.AluOpType.mult,
            )
            for m in range(1, M):
                src, dst = wacc[(m + 1) % 2], wacc[m % 2]
                ins = dve.scalar_tensor_tensor(
                    dst[:, :], mats_sb[:, m * T : (m + 1) * T], bc[:, m : m + 1],
                    src[:, :], mybir.AluOpType.mult, mybir.AluOpType.add,
                )
                if m == M - 1:
                    ins.then_inc(s_w, 1)
            for p, s in enumerate(PAIR_SIZES):
                dve.wait_ge(s_sig, p + 1)
                if p >= SLOTS:
                    # output DMA of pair p-SLOTS must be done before reuse
                    dve.wait_ge(s_out, 16 * (p - SLOTS + 1))
                # prune: keep sig where sig >= 0.1 (== sigmoid(z+b) >= 0.1)
                ss = sig[p % SLOTS]
                dve.scalar_tensor_tensor(
                    adj[p % SLOTS][:, 0:s], ss[:, 0:s], CROSS_PRUNE, ss[:, 0:s],
                    mybir.AluOpType.is_ge, mybir.AluOpType.mult,
                ).then_inc(s_adjv, 1)

        @block.scalar
        def _(act):
            act.wait_ge(s_bc, 1)
            bcol = bc[:, M : M + 1]
            for p, s in enumerate(PAIR_SIZES):
                act.wait_ge(s_mm, 2 * p + 2)
                if p >= SLOTS:
                    # DVE must be done reading sig slot (stt of pair p-SLOTS)
                    act.wait_ge(s_adjv, p - SLOTS + 1)
                act.activation(
                    sig[p % SLOTS][:, 0:s], z[p % SLOTS][:, 0:s],
                    mybir.ActivationFunctionType.Sigmoid, bias=bcol,
                ).then_inc(s_sig, 1)
                # issue output DMA for the previous pair (adj ready by now)
                if p >= 1:
                    q, sq = p - 1, PAIR_SIZES[p - 1]
                    act.wait_ge(s_adjv, q + 1)
                    act.dma_start(
                        out=out[:, poffs[q] : poffs[q] + sq],
                        in_=adj[q % SLOTS][:, 0:sq],
                    ).then_inc(s_out, 16)
            q, sq = NPAIR - 1, PAIR_SIZES[-1]
            act.wait_ge(s_adjv, q + 1)
            act.dma_start(
                out=out[:, poffs[q] : poffs[q] + sq],
                in_=adj[q % SLOTS][:, 0:sq],
            ).then_inc(s_out, 16)

    return nc


_CACHED_NC = None


def make_in_maps(x, mats, head_w, head_b):
    x = np.ascontiguousarray(x, dtype=np.float32)
    mats = np.ascontiguousarray(mats, dtype=np.float32)
    head_w = np.asarray(head_w, dtype=np.float32)
    head_b = np.asarray(head_b, dtype=np.float32)

    xT = np.ascontiguousarray(x.T)  # [D, N]
    matsT = np.ascontiguousarray(mats.transpose(0, 2, 1))  # [M, D, T]
    hwb = np.concatenate(
        [head_w.reshape(M), head_b.reshape(1), np.ones(D, np.float32)]
    ).reshape(1, M + 1 + D).astype(np.float32)

    return [
        {
            "xT": np.ascontiguousarray(xT[:, c * NSH : (c + 1) * NSH]),
            "matsT": matsT,
            "headwb": hwb,
        }
        for c in range(N_CORES)
    ]


def unpack_out(results):
    out = np.empty((T, N), dtype=np.float32)
    for c in range(N_CORES):
        packed = results[c]["out"]  # [128, 6250]
        base = c * NSH
        xoff = 0
        poff = 0
        for s in PAIR_SIZES:
            out[:, base + xoff : base + xoff + s] = packed[0:T, poff : poff + s]
            out[:, base + xoff + s : base + xoff + 2 * s] = packed[T:D, poff : poff + s]
            xoff += 2 * s
            poff += s
    return out


def kernel(x, mats, head_w, head_b):
    global _CACHED_NC
    if _CACHED_NC is None:
        _CACHED_NC = build_nc()
    nc = _CACHED_NC

    in_maps = make_in_maps(x, mats, head_w, head_b)
    results = run_bass_kernel_spmd(nc, in_maps, core_ids=list(range(N_CORES))).results
    return unpack_out(results)


# revision 18
# speedup vs baseline: 6992.2997x; 6992.2997x over previous
"""Trainium2 Bass kernel for fused cross-adjacency:
    w = einsum('m,mtd->td', head_w, mats); z = w @ x.T + head_b
    out = where(sigmoid(z) < 0.1, 0, sigmoid(z))           # [T=64, N=100000]

Sharding: node dim N split across 8 cores (12500 nodes each); tiny params
replicated. Host feeds x pre-transposed ([D=128, N/8] per core) so the
contraction dim D lands on SBUF partitions with no on-chip transpose.

Per chunk pair (2 x s columns): one input DMA brings [128, 2s] of xT; two
col-tiled matmuls (out partitions 0:64 / 64:128 of one PSUM bank) compute z
for both chunks; ScalarE applies sigmoid(z + b) with the bias folded into
the activation; VectorE applies the prune (sig >= 0.1) * sig in one
scalar_tensor_tensor; one output DMA stores the packed [128, s] tile.
Output DRAM is a packed [128, 6250] layout (two T=64 row-halves per column
block), unpacked on host. Raw Bass with a 4-slot ring pipeline: input DMA
on the SP queue, output DMA on the Activation HWDGE queue, so input/output
transfers ride different queues.
"""

import contextlib
import numpy as np

import concourse.bass as bass
import concourse.mybir as mybir
from concourse.bass_utils import run_bass_kernel_spmd

N, T, D, M = 100000, 64, 128, 8
N_CORES = 8
NSH = N // N_CORES  # 12500
CROSS_PRUNE = 0.1

# pair p processes two consecutive chunks of s columns each; chunk A goes to
# packed rows 0:64, chunk B to rows 64:128, at packed columns [poff, poff+s).
PAIR_SIZES = [500] * 12 + [250]
PACKED_W = sum(PAIR_SIZES)  # 6250
assert 2 * PACKED_W == NSH

SLOTS = 4  # ring depth for xt / z / sig / adj
SLOT_W = max(PAIR_SIZES)

F32 = mybir.dt.float32
NPAIR = len(PAIR_SIZES)


def build_nc(reps=1):
    """reps > 1 unrolls the whole main loop `reps` times over the same data
    (used only for timing: the per-rep slope isolates on-device exec time
    from dispatch overhead)."""
    nc = bass.Bass()
    xT = nc.declare_dram_parameter("xT", [D, NSH], F32, isOutput=False)
    matsT = nc.declare_dram_parameter("matsT", [M, D, T], F32, isOutput=False)
    # [head_w(8), head_b(1), ones(128)] in one row
    headwb = nc.declare_dram_parameter("headwb", [1, M + 1 + D], F32, isOutput=False)
    out = nc.declare_dram_parameter("out", [D, PACKED_W], F32, isOutput=True)

    ctx = contextlib.ExitStack()
    with ctx:
        hwb = ctx.enter_context(nc.sbuf_tensor("hwb", [1, M + 1 + D], F32))
        bc = ctx.enter_context(nc.sbuf_tensor("bc", [D, M + 1], F32))
        mats_sb = ctx.enter_context(nc.sbuf_tensor("mats_sb", [D, M * T], F32))
        w0 = ctx.enter_context(nc.sbuf_tensor("w0", [D, T], F32))
        w1 = ctx.enter_context(nc.sbuf_tensor("w1", [D, T], F32))
        xt = [
            ctx.enter_context(nc.sbuf_tensor(f"xt{i}", [D, 2 * SLOT_W], F32))
            for i in range(SLOTS)
        ]
        sig = [
            ctx.enter_context(nc.sbuf_tensor(f"sig{i}", [D, SLOT_W], F32))
            for i in range(SLOTS)
        ]
        adj = [
            ctx.enter_context(nc.sbuf_tensor(f"adj{i}", [D, SLOT_W], F32))
            for i in range(SLOTS)
        ]
        bc_ps = ctx.enter_context(nc.psum_tensor("bc_ps", [D, M + 1], F32))
        z = [
            ctx.enter_context(nc.psum_tensor(f"z{i}", [D, SLOT_W], F32))
            for i in range(SLOTS)
        ]

        s_hwb = ctx.enter_context(nc.semaphore("s_hwb"))
        s_mats = ctx.enter_context(nc.semaphore("s_mats"))
        s_pe_pre = ctx.enter_context(nc.semaphore("s_pe_pre"))
        s_bc = ctx.enter_context(nc.semaphore("s_bc"))
        s_w = ctx.enter_context(nc.semaphore("s_w"))
        s_x = ctx.enter_context(nc.semaphore("s_x"))
        s_mm = ctx.enter_context(nc.semaphore("s_mm"))
        s_sig = ctx.enter_context(nc.semaphore("s_sig"))
        s_adjv = ctx.enter_context(nc.semaphore("s_adjv"))
        s_out = ctx.enter_context(nc.semaphore("s_out"))

        wacc = [w0, w1]
        wT = wacc[(M - 1) % 2]

        xoffs = []
        poffs = []
        xo = po = 0
        for s in PAIR_SIZES:
            xoffs.append(xo)
            poffs.append(po)
            xo += 2 * s
            po += s
        pairs = PAIR_SIZES * reps
        xoffs = xoffs * reps
        poffs = poffs * reps
        npair = len(pairs)

        block = ctx.enter_context(nc.Block())

        @block.sync
        def _(sync):
            sync.dma_start(out=hwb[:, :], in_=headwb[:, :]).then_inc(s_hwb, 16)
            for m in range(M):
                sync.dma_start(
                    out=mats_sb[:, m * T : (m + 1) * T], in_=matsT[m, :, :]
                ).then_inc(s_mats, 16)
            for p, s in enumerate(pairs):
                if p >= SLOTS:
                    # PE must be done reading xt slot (mm2 of pair p-SLOTS)
                    sync.wait_ge(s_mm, 2 * (p - SLOTS) + 2)
                sync.dma_start(
                    out=xt[p % SLOTS][:, 0 : 2 * s],
                    in_=xT[:, xoffs[p] : xoffs[p] + 2 * s],
                ).then_inc(s_x, 16)

        @block.tensor
        def _(pe):
            pe.wait_ge(s_hwb, 16)
            # broadcast head_w/head_b to all 128 partitions: ones^T @ [hw|hb]
            pe.matmul(
                bc_ps[:, :], hwb[:, M + 1 :], hwb[:, 0 : M + 1],
                start=True, stop=True,
            ).then_inc(s_pe_pre, 1)
            pe.wait_ge(s_w, 1)
            for p, s in enumerate(pairs):
                pe.wait_ge(s_x, 16 * (p + 1))
                if p >= SLOTS:
                    # ACT must be done reading z slot (sigmoid of pair p-SLOTS)
                    pe.wait_ge(s_sig, p - SLOTS + 1)
                zz = z[p % SLOTS]
                xx = xt[p % SLOTS]
                pe.matmul(
                    zz[0:T, 0:s], wT[:, :], xx[:, 0:s], start=True, stop=True
                ).then_inc(s_mm, 1)
                pe.matmul(
                    zz[T:D, 0:s], wT[:, :], xx[:, s : 2 * s],
                    start=True, stop=True,
                ).then_inc(s_mm, 1)

        @block.vector
        def _(dve):
            dve.wait_ge(s_pe_pre, 1)
            dve.tensor_copy(bc[:, :], bc_ps[:, :]).then_inc(s_bc, 1)
            dve.wait_ge(s_mats, 16 * M)
            # wT[d, t] = sum_m head_w[m] * matsT[m, d, t]
            dve.tensor_scalar(
                wacc[0][:, :], mats_sb[:, 0:T], bc[:, 0:1], None,
                mybir.AluOpType.mult,
            )
            for m in range(1, M):
                src, dst = wacc[(m + 1) % 2], wacc[m % 2]
                ins = dve.scalar_tensor_tensor(
                    dst[:, :], mats_sb[:, m * T : (m + 1) * T], bc[:, m : m + 1],
                    src[:, :], mybir.AluOpType.mult, mybir.AluOpType.add,
                )
                if m == M - 1:
                    ins.then_inc(s_w, 1)
            for p, s in enumerate(pairs):
                dve.wait_ge(s_sig, p + 1)
                if p >= SLOTS:
                    # output DMA of pair p-SLOTS must be done before reuse
                    dve.wait_ge(s_out, 16 * (p - SLOTS + 1))
                # prune: keep sig where sig >= 0.1 (== sigmoid(z+b) >= 0.1)
                ss = sig[p % SLOTS]
                dve.scalar_tensor_tensor(
                    adj[p % SLOTS][:, 0:s], ss[:, 0:s], CROSS_PRUNE, ss[:, 0:s],
                    mybir.AluOpType.is_ge, mybir.AluOpType.mult,
                ).then_inc(s_adjv, 1)

        @block.scalar
        def _(act):
            act.wait_ge(s_bc, 1)
            bcol = bc[:, M : M + 1]
            for p, s in enumerate(pairs):
                act.wait_ge(s_mm, 2 * p + 2)
                if p >= SLOTS:
                    # DVE must be done reading sig slot (stt of pair p-SLOTS)
                    act.wait_ge(s_adjv, p - SLOTS + 1)
                act.activation(
                    sig[p % SLOTS][:, 0:s], z[p % SLOTS][:, 0:s],
                    mybir.ActivationFunctionType.Sigmoid, bias=bcol,
                ).then_inc(s_sig, 1)
                # issue output DMA for the previous pair (adj ready by now)
                if p >= 1:
                    q, sq = p - 1, pairs[p - 1]
                    act.wait_ge(s_adjv, q + 1)
                    act.dma_start(
                        out=out[:, poffs[q] : poffs[q] + sq],
                        in_=adj[q % SLOTS][:, 0:sq],
                    ).then_inc(s_out, 16)
            q, sq = npair - 1, pairs[-1]
            act.wait_ge(s_adjv, q + 1)
            act.dma_start(
                out=out[:, poffs[q] : poffs[q] + sq],
                in_=adj[q % SLOTS][:, 0:sq],
            ).then_inc(s_out, 16)

    return nc


_CACHED_NC = None


def make_in_maps(x, mats, head_w, head_b):
    x = np.ascontiguousarray(x, dtype=np.float32)
    mats = np.ascontiguousarray(mats, dtype=np.float32)
    head_w = np.asarray(head_w, dtype=np.float32)
    head_b = np.asarray(head_b, dtype=np.float32)

    xT = np.ascontiguousarray(x.T)  # [D, N]
    matsT = np.ascontiguousarray(mats.transpose(0, 2, 1))  # [M, D, T]
    hwb = np.concatenate(
        [head_w.reshape(M), head_b.reshape(1), np.ones(D, np.float32)]
    ).reshape(1, M + 1 + D).astype(np.float32)

    return [
        {
            "xT": np.ascontiguousarray(xT[:, c * NSH : (c + 1) * NSH]),
            "matsT": matsT,
            "headwb": hwb,
        }
        for c in range(N_CORES)
    ]


def unpack_out(results):
    out = np.empty((T, N), dtype=np.float32)
    for c in range(N_CORES):
        packed = results[c]["out"]  # [128, 6250]
        base = c * NSH
        xoff = 0
        poff = 0
        for s in PAIR_SIZES:
            out[:, base + xoff : base + xoff + s] = packed[0:T, poff : poff + s]
            out[:, base + xoff + s : base + xoff + 2 * s] = packed[T:D, poff : poff + s]
            xoff += 2 * s
            poff += s
    return out


def kernel(x, mats, head_w, head_b):
    global _CACHED_NC
    if _CACHED_NC is None:
        _CACHED_NC = build_nc()
    nc = _CACHED_NC

    in_maps = make_in_maps(x, mats, head_w, head_b)
    results = run_bass_kernel_spmd(nc, in_maps, core_ids=list(range(N_CORES))).results
    return unpack_out(results)
